# revision 1
# baseline (speedup 1.0000x reference)
"""Trainium2 Bass kernel for nn_DeepseekLayer (dense transformer layer).

Sharding (8 cores): Megatron-style TP.
  - attention: head-sharded (2 heads/core); q/k/v projections over head shards;
    transposed-softmax layout (scores [sk, sq]); AllToAll switches attention
    output to token shards so o_proj needs no all-reduce.
  - o_proj + residual + rmsnorm2: token-sharded (256 tokens/core).
  - MLP: AllGather hidden -> tensor-parallel gate/up/down (1024 ff dims/core)
    -> chunked ReduceScatter (overlapped with down) -> local residual add ->
    host gathers token shards.
All heavy matmuls run in float32r (fp32 bits rounded to 11 mantissa bits,
1 PE cycle/row). Weights are pre-transposed/pre-tiled/pre-rounded on host.
rmsnorm1 is folded into rope tables / V-copy scales (scaling commutes with
the linear projections), so qkv runs directly on the raw (rounded) input.
"""
import numpy as np
from contextlib import ExitStack

from concourse import bacc
import concourse.tile as tile
import concourse.mybir as mybir
from concourse.bass_utils import run_bass_kernel_spmd

F32 = mybir.dt.float32
F32R = mybir.dt.float32r
AF = mybir.ActivationFunctionType
OP = mybir.AluOpType

H = 2048          # hidden
NH = 16           # heads
HD = 128          # head dim
MLP = 8192
S = 2048          # sequence
B = 1
EPS = 1e-6
NC = 8            # cores
HPC = NH // NC    # heads per core = 2
EH = HPC * HD     # qkv out dims per core = 256
MSH = MLP // NC   # mlp dims per core = 1024
SSH = S // NC     # tokens per shard = 256
RG = [list(range(NC))]
DT = H // 128     # 16 d-tiles
MT = MSH // 128   # 8 m-tiles per core


def round_fp32r(x: np.ndarray) -> np.ndarray:
    """Round fp32 to fp32r (11 mantissa bits, RNE) — matches walrus fp32_to_fp32r."""
    u = np.ascontiguousarray(x, dtype=np.float32).view(np.uint32).astype(np.uint64)
    r = (u + 0x7FF + ((u >> 12) & 1)) & 0xFFFFF000
    return r.astype(np.uint32).view(np.float32)


_LDW_PATCHED = False


def _enable_ldw_opt():
    """Compile our NEFF with walrus --enable-ldw-opt=true (elides redundant
    LDWEIGHTS; concourse's default is false)."""
    global _LDW_PATCHED
    if _LDW_PATCHED:
        return
    import concourse.bass_utils as _bu
    _orig = _bu.run_command

    def _patched(argv, **kw):
        argv = ["--enable-ldw-opt=true" if a == "--enable-ldw-opt=false" else a
                for a in argv]
        return _orig(argv, **kw)

    _bu.run_command = _patched
    _LDW_PATCHED = True


def _build_program():
    _enable_ldw_opt()
    nc = bacc.Bacc(trn_type="TRN2", target_bir_lowering=False, debug=False,
                   num_devices=NC)

    def inp(name, shape, dt):
        return nc.dram_tensor(name, shape, dt, kind="ExternalInput").ap()

    xTr = inp("xTr", [H, S], F32R)              # round_fp32r(x).T (feature-major)
    xTrs = inp("xTrs", [H, SSH], F32R)          # this core's token-shard of xTr
    cosT = inp("cosT", [HD, S], F32)
    sinT = inp("sinT", [HD, S], F32)
    wqT = inp("wqT", [H, EH], F32R)             # (wq*n1w/sqrt(HD)).T shard
    wkT = inp("wkT", [H, EH], F32R)             # (wk*n1w).T shard
    wvT = inp("wvT", [H, EH], F32R)             # (wv*n1w).T shard
    woTt = inp("woTt", [128, DT, DT, 128], F32R)   # wo.T tiled [p, et, dt, c]
    wgTt = inp("wgTt", [128, DT, MT, 128], F32R)   # (wg*n2w).T shard tiled [p, dt, mt, c]
    wuTt = inp("wuTt", [128, DT, MT, 128], F32R)
    wdTt = inp("wdTt", [128, MT, DT, 128], F32R)   # wd shard.T tiled [p, mt, dt, c]
    out_sh = nc.dram_tensor("out_sh", [H, SSH], F32, kind="ExternalOutput").ap()

    with tile.TileContext(nc) as tc, ExitStack() as top:
        dram = top.enter_context(tc.tile_pool(name="dram", bufs=1, space="DRAM"))
        per = top.enter_context(tc.tile_pool(name="per", bufs=1))
        ones_f = per.tile([128, 1], F32)
        nc.gpsimd.memset(ones_f[:], 1.0)
        ones_r = per.tile([128, 1], F32R)
        nc.vector.tensor_copy(ones_r[:], ones_f[:])
        ones_row = per.tile([1, 128], F32)
        nc.gpsimd.memset(ones_row[:], 1.0)
        one_s = per.tile([1, 1], F32)
        nc.gpsimd.memset(one_s[:], 1.0)
        eps1 = per.tile([1, 1], F32)
        nc.gpsimd.memset(eps1[:], EPS)
        from concourse.masks import make_identity
        ident_f = per.tile([128, 128], F32)
        make_identity(nc, ident_f[:])
        ident_r = per.tile([128, 128], F32R)
        nc.vector.tensor_copy(ident_r[:], ident_f[:])

        qk_ctx = ExitStack()
        qk_pool = qk_ctx.enter_context(tc.tile_pool(name="qk", bufs=1))
        qr = [qk_pool.tile([128, S], F32R, name=f"qr{h}") for h in range(HPC)]
        kr = [qk_pool.tile([128, S], F32R, name=f"kr{h}") for h in range(HPC)]
        V_sb = qk_pool.tile([128, S // 128, EH], F32R, name="V_sb")
        att = [qk_pool.tile([128, S], F32R, name=f"att{h}") for h in range(HPC)]

        s12_ctx = ExitStack()
        s12 = s12_ctx.enter_context(tc.tile_pool(name="s12", bufs=1))
        cos_sb = s12.tile([HD, S], F32, name="cos_sb")
        sin_sb = s12.tile([HD, S], F32, name="sin_sb")
        wq_sb = s12.tile([128, DT, EH], F32R, name="wq_sb")
        wk_sb = s12.tile([128, DT, EH], F32R, name="wk_sb")
        wv_sb = s12.tile([128, DT, EH], F32R, name="wv_sb")

        # ---- S12: fused rmsnorm1 stats + qkv + rope + V (per s-quarter) ----
        # norm scaling commutes with the projections:
        #   q_normed = rstd[s] * (x @ wq.T)  -> fold rstd into rope cos/sin
        #   v_normed = rstd[s] * v           -> fold rstd into the V PSUM copy
        with tc.tile_pool(name="s2", bufs=1) as s2, \
             tc.tile_pool(name="ps2", bufs=1, space="PSUM") as ps2:
            # pass 1+2 (s-halves): v projection + sumsq stats; pass 3+4: q/k.
            # xTr is streamed per pass; weights stay stationary across the two
            # 512-wide chunks of each half (1 LDWEIGHTS per 2 matmuls).
            rstd_bc = [None, None, None, None]
            for half in range(2):
                hb = half * 1024
                ss_ps = [ps2.tile([1, 512], F32, tag=f"pp{i}", name=f"ss{i}", bufs=1)
                         for i in range(2)]
                v_ps = [[ps2.tile([128, 512], F32, tag=f"pp{2 + h * 2 + i}",
                                  name=f"v{h}{i}", bufs=1)
                         for i in range(2)] for h in range(HPC)]
                for dt in range(DT):
                    xt = s2.tile([128, 1024], F32R, tag="xv", name="xv", bufs=3)
                    nc.sync.dma_start(xt[:], xTr[dt * 128:(dt + 1) * 128, hb:hb + 1024])
                    if half == 0:
                        nc.sync.dma_start(wv_sb[:, dt, :], wvT[dt * 128:(dt + 1) * 128, :])
                    x2 = s2.tile([128, 1024], F32R, tag="x2", bufs=2)
                    nc.vector.tensor_tensor(out=x2[:], in0=xt[:], in1=xt[:], op=OP.mult)
                    for i in range(2):
                        nc.tensor.matmul(ss_ps[i][:], ones_r[:],
                                         x2[:, i * 512:(i + 1) * 512],
                                         start=(dt == 0), stop=(dt == DT - 1))
                    for h in range(HPC):
                        for i in range(2):
                            nc.tensor.matmul(v_ps[h][i][:],
                                             wv_sb[:, dt, h * 128:(h + 1) * 128],
                                             xt[:, i * 512:(i + 1) * 512],
                                             start=(dt == 0), stop=(dt == DT - 1))
                # rstd for both quarters of this half
                for i in range(2):
                    qd = half * 2 + i
                    ssq = s2.tile([1, 512], F32, tag="ssq", bufs=2)
                    nc.scalar.activation(ssq[:], ss_ps[i][:], AF.Sqrt, bias=eps1[:],
                                         scale=1.0 / H)
                    rstd = s2.tile([1, 512], F32, tag="rstd", bufs=4)
                    nc.vector.reciprocal(rstd[:], ssq[:])
                    bc_ps = ps2.tile([128, 512], F32, tag=f"pp{i}", name="bc_ps", bufs=1)
                    nc.tensor.matmul(bc_ps[:], ones_row[:], rstd[:], start=True, stop=True)
                    rb = s2.tile([128, 512], F32, tag=f"rstd_bc{qd}", name=f"rb{qd}",
                                 bufs=1)
                    nc.vector.tensor_copy(rb[:], bc_ps[:])
                    rstd_bc[qd] = rb
                # V: scale vT by rstd, then PE-transpose to token-major V_sb
                for h in range(HPC):
                    for i in range(2):
                        qd = half * 2 + i
                        vsc = s2.tile([128, 512], F32R, tag="vsc", bufs=2)
                        nc.vector.tensor_tensor(out=vsc[:], in0=v_ps[h][i][:],
                                                in1=rstd_bc[qd][:], op=OP.mult)
                        for sti in range(4):
                            st = qd * 4 + sti
                            tr_ps = ps2.tile([128, 128], F32R, tag=f"pp{6 + h}",
                                             name="tr_ps", bufs=1)
                            nc.tensor.transpose(tr_ps[:],
                                                vsc[:, sti * 128:(sti + 1) * 128],
                                                ident_r[:])
                            nc.vector.tensor_copy(V_sb[:, st, h * 128:(h + 1) * 128],
                                                  tr_ps[:])
                if half == 0:
                    # overlap q/k weight loads + rope tables with pass-2 compute
                    for dt in range(DT):
                        nc.sync.dma_start(wq_sb[:, dt, :], wqT[dt * 128:(dt + 1) * 128, :])
                        nc.sync.dma_start(wk_sb[:, dt, :], wkT[dt * 128:(dt + 1) * 128, :])
                    nc.sync.dma_start(cos_sb[:], cosT)
                    nc.sync.dma_start(sin_sb[:], sinT)
            for half in range(2):
                hb = half * 1024
                q_ps = [[ps2.tile([128, 512], F32, tag=f"pp{h * 2 + i}",
                                  name=f"q{h}{i}", bufs=1)
                         for i in range(2)] for h in range(HPC)]
                k_ps = [[ps2.tile([128, 512], F32, tag=f"pp{4 + h * 2 + i}",
                                  name=f"k{h}{i}", bufs=1)
                         for i in range(2)] for h in range(HPC)]
                for dt in range(DT):
                    xt = s2.tile([128, 1024], F32R, tag="xv", name="xv2", bufs=3)
                    nc.sync.dma_start(xt[:], xTr[dt * 128:(dt + 1) * 128, hb:hb + 1024])
                    for h in range(HPC):
                        for i in range(2):
                            nc.tensor.matmul(q_ps[h][i][:],
                                             wq_sb[:, dt, h * 128:(h + 1) * 128],
                                             xt[:, i * 512:(i + 1) * 512],
                                             start=(dt == 0), stop=(dt == DT - 1))
                        for i in range(2):
                            nc.tensor.matmul(k_ps[h][i][:],
                                             wk_sb[:, dt, h * 128:(h + 1) * 128],
                                             xt[:, i * 512:(i + 1) * 512],
                                             start=(dt == 0), stop=(dt == DT - 1))
                for i in range(2):
                    qd = half * 2 + i
                    c0 = qd * 512
                    cs_c = s2.tile([HD, 512], F32, tag="cs_c", bufs=2)
                    nc.vector.tensor_tensor(out=cs_c[:], in0=cos_sb[:, c0:c0 + 512],
                                            in1=rstd_bc[qd][:], op=OP.mult)
                    cs_s = s2.tile([HD, 512], F32, tag="cs_s", bufs=2)
                    nc.vector.tensor_tensor(out=cs_s[:], in0=sin_sb[:, c0:c0 + 512],
                                            in1=rstd_bc[qd][:], op=OP.mult)
                    for h in range(HPC):
                        for (src_ps, dst) in ((q_ps[h][i], qr[h]), (k_ps[h][i], kr[h])):
                            m1 = s2.tile([64, 512], F32, tag="m1", bufs=1)
                            m2 = s2.tile([64, 512], F32, tag="m2", bufs=1)
                            nc.vector.tensor_tensor(out=m1[:], in0=src_ps[0:64, :],
                                                    in1=cs_c[0:64, :], op=OP.mult)
                            nc.vector.tensor_tensor(out=m2[:], in0=src_ps[64:128, :],
                                                    in1=cs_s[0:64, :], op=OP.mult)
                            nc.vector.tensor_tensor(out=dst[0:64, c0:c0 + 512],
                                                    in0=m1[:], in1=m2[:], op=OP.subtract)
                            m3 = s2.tile([64, 512], F32, tag="m3", bufs=1)
                            m4 = s2.tile([64, 512], F32, tag="m4", bufs=1)
                            nc.vector.tensor_tensor(out=m3[:], in0=src_ps[64:128, :],
                                                    in1=cs_c[64:128, :], op=OP.mult)
                            nc.vector.tensor_tensor(out=m4[:], in0=src_ps[0:64, :],
                                                    in1=cs_s[64:128, :], op=OP.mult)
                            nc.vector.tensor_tensor(out=dst[64:128, c0:c0 + 512],
                                                    in0=m3[:], in1=m4[:], op=OP.add)

        # ---- S3: attention (transposed softmax, no max subtraction) ----
        s12_ctx.close()
        a2a_in = [dram.tile([NC, 128, SSH], F32R, name=f"a2a_in{h}") for h in range(HPC)]
        a2a_out = [dram.tile([NC, 128, SSH], F32R, name=f"a2a_out{h}") for h in range(HPC)]
        with tc.tile_pool(name="s3", bufs=1) as s3, \
             tc.tile_pool(name="ps3", bufs=1, space="PSUM") as ps3:
            for h in range(HPC):
                for scp in range(2):     # pairs of sq-512 chunks (lhsT reuse x2)
                    q0 = scp * 1024
                    q1 = q0 + 512
                    av0 = ps3.tile([128, 512], F32, tag="av0", name="av0", bufs=1)
                    av1 = ps3.tile([128, 512], F32, tag="av1", name="av1", bufs=1)
                    sm0 = ps3.tile([1, 512], F32, tag="sm0", name="sm0", bufs=1)
                    sm1 = ps3.tile([1, 512], F32, tag="sm1", name="sm1", bufs=1)
                    for kt in range(DT):  # sk tiles of 128
                        st0 = ps3.tile([128, 512], F32, tag="st", name="st0", bufs=2)
                        nc.tensor.matmul(st0[:], kr[h][:, kt * 128:(kt + 1) * 128],
                                         qr[h][:, q0:q0 + 512], start=True, stop=True)
                        st1 = ps3.tile([128, 512], F32, tag="st", name="st1", bufs=2)
                        nc.tensor.matmul(st1[:], kr[h][:, kt * 128:(kt + 1) * 128],
                                         qr[h][:, q1:q1 + 512], start=True, stop=True)
                        e0 = s3.tile([128, 512], F32R, tag="e", bufs=4)
                        nc.scalar.activation(e0[:], st0[:], AF.Exp)
                        e1 = s3.tile([128, 512], F32R, tag="e", bufs=4)
                        nc.scalar.activation(e1[:], st1[:], AF.Exp)
                        nc.tensor.matmul(sm0[:], ones_r[:], e0[:],
                                         start=(kt == 0), stop=(kt == DT - 1))
                        nc.tensor.matmul(sm1[:], ones_r[:], e1[:],
                                         start=(kt == 0), stop=(kt == DT - 1))
                        nc.tensor.matmul(av0[:], V_sb[:, kt, h * 128:(h + 1) * 128],
                                         e0[:], start=(kt == 0), stop=(kt == DT - 1))
                        nc.tensor.matmul(av1[:], V_sb[:, kt, h * 128:(h + 1) * 128],
                                         e1[:], start=(kt == 0), stop=(kt == DT - 1))
                    for (qq, sm, av) in ((q0, sm0, av0), (q1, sm1, av1)):
                        rs_sb = s3.tile([1, 512], F32, tag="rs", bufs=2)
                        nc.vector.reciprocal(rs_sb[:], sm[:])
                        bc_ps = ps3.tile([128, 512], F32, tag="bc", name="bc_ps3", bufs=2)
                        nc.tensor.matmul(bc_ps[:], ones_row[:], rs_sb[:],
                                         start=True, stop=True)
                        bc_sb = s3.tile([128, 512], F32, tag="bcs", bufs=2)
                        nc.vector.tensor_copy(bc_sb[:], bc_ps[:])
                        nc.vector.tensor_tensor(out=att[h][:, qq:qq + 512], in0=av[:],
                                                in1=bc_sb[:], op=OP.mult)
                # ship this head's attention output while the next head computes
                for j in range(NC):
                    nc.sync.dma_start(a2a_in[h][j], att[h][:, j * SSH:(j + 1) * SSH])
                nc.gpsimd.collective_compute("AllToAll", OP.bypass,
                                             ins=[a2a_in[h][:]], outs=[a2a_out[h][:]],
                                             replica_groups=RG)

        # ---- S4: AllToAll (split per head) to token shards + o_proj + residual ----
        qk_ctx.close()
        res_pool = top.enter_context(tc.tile_pool(name="res", bufs=1))
        res1 = [res_pool.tile([128, SSH], F32, name=f"res1_{dt}") for dt in range(DT)]
        with tc.tile_pool(name="s4", bufs=1) as s4, \
             tc.tile_pool(name="ps4", bufs=1, space="PSUM") as ps4:
            attg = s4.tile([128, DT, SSH], F32R, tag="attg")
            for et in range(DT):
                nc.sync.dma_start(attg[:, et, :], a2a_out[et % 2][et // 2])
            for dt in range(DT):
                wo_t = s4.tile([128, DT, 128], F32R, tag="wo", bufs=2)
                nc.sync.dma_start(wo_t[:], woTt[:, :, dt, :])
                o_ps = ps4.tile([128, SSH], F32, tag="o", name="o_ps", bufs=2)
                for et in range(DT):
                    nc.tensor.matmul(o_ps[:], wo_t[:, et, :], attg[:, et, :],
                                     start=(et == 0), stop=(et == DT - 1))
                xs = s4.tile([128, SSH], F32R, tag="xs", bufs=2)
                nc.sync.dma_start(xs[:], xTrs[dt * 128:(dt + 1) * 128, :])
                nc.vector.tensor_tensor(out=res1[dt][:], in0=o_ps[:], in1=xs[:], op=OP.add)

        # ---- S5: rmsnorm2 on token shard ----
        h2_ctx = ExitStack()
        h2p = h2_ctx.enter_context(tc.tile_pool(name="h2p", bufs=1))
        h2 = [h2p.tile([128, SSH], F32R, name=f"h2_{dt}") for dt in range(DT)]
        with tc.tile_pool(name="s5", bufs=1) as s5, \
             tc.tile_pool(name="ps5", bufs=1, space="PSUM") as ps5:
            ss2_ps = ps5.tile([1, SSH], F32, tag="ss2", name="ss2_ps")
            for dt in range(DT):
                x2 = s5.tile([128, SSH], F32R, tag="x22", bufs=2)
                nc.vector.tensor_tensor(out=x2[:], in0=res1[dt][:], in1=res1[dt][:],
                                        op=OP.mult)
                nc.tensor.matmul(ss2_ps[:], ones_r[:], x2[:],
                                 start=(dt == 0), stop=(dt == DT - 1))
            ssq2 = s5.tile([1, SSH], F32, tag="ssq2")
            nc.scalar.activation(ssq2[:], ss2_ps[:], AF.Sqrt, bias=eps1[:], scale=1.0 / H)
            rstd2 = s5.tile([1, SSH], F32, tag="rstd2")
            nc.vector.reciprocal(rstd2[:], ssq2[:])
            bc2_ps = ps5.tile([128, SSH], F32, tag="bc2", name="bc2_ps", bufs=1)
            nc.tensor.matmul(bc2_ps[:], ones_row[:], rstd2[:], start=True, stop=True)
            rstd2_bc = s5.tile([128, SSH], F32, tag="rstd2bc")
            nc.vector.tensor_copy(rstd2_bc[:], bc2_ps[:])
            for dt in range(DT):
                nc.vector.tensor_tensor(out=h2[dt][:], in0=res1[dt][:],
                                        in1=rstd2_bc[:], op=OP.mult)

        # ---- S6: AllGather hidden shards (split in two d-halves) ----
        ag_in = [dram.tile([H // 2, SSH], F32R, name=f"ag_in{i}") for i in range(2)]
        ag_out = [dram.tile([NC, H // 2, SSH], F32R, addr_space="Shared",
                            name=f"ag_out{i}") for i in range(2)]
        for i in range(2):
            for k in range(DT // 2):
                dt = i * (DT // 2) + k
                nc.sync.dma_start(ag_in[i][k * 128:(k + 1) * 128, :], h2[dt][:])
            nc.gpsimd.collective_compute("AllGather", OP.bypass,
                                         ins=[ag_in[i][:]], outs=[ag_out[i][:]],
                                         replica_groups=RG)
        h2_ctx.close()

        # ---- S7: MLP gate/up (per s-half), then full-s down + chunked RS ----
        rs_in = [dram.tile([NC, 512, SSH], F32, name=f"rs_in{g}") for g in range(4)]
        rs_out = [dram.tile([512, SSH], F32, name=f"rs_out{g}") for g in range(4)]
        with tc.tile_pool(name="s7", bufs=1) as s7, \
             tc.tile_pool(name="ps7", bufs=1, space="PSUM") as ps7:
            act_t = [s7.tile([128, S], F32R, tag=f"act{mt}", name=f"act{mt}", bufs=1)
                     for mt in range(MT)]
            for half in range(2):        # s halves of 1024
                h2g = []
                for dt in range(DT):
                    t = s7.tile([128, 1024], F32R, tag=f"hg{dt}", name=f"hg{dt}", bufs=1)
                    gi, gr = (0, dt) if dt < DT // 2 else (1, dt - DT // 2)
                    for k in range(4):
                        r = half * 4 + k
                        nc.sync.dma_start(t[:, k * 256:(k + 1) * 256],
                                          ag_out[gi][r, gr * 128:(gr + 1) * 128, :])
                    h2g.append(t)
                hb = half * 1024
                for mt in range(MT):
                    wg_t = s7.tile([128, DT, 128], F32R, tag="wg", bufs=1)
                    wu_t = s7.tile([128, DT, 128], F32R, tag="wu", bufs=1)
                    nc.sync.dma_start(wg_t[:], wgTt[:, :, mt, :])
                    nc.sync.dma_start(wu_t[:], wuTt[:, :, mt, :])
                    g_ps = [ps7.tile([128, 512], F32, tag=f"g{i}", name=f"g{i}", bufs=1)
                            for i in range(2)]
                    u_ps = [ps7.tile([128, 512], F32, tag=f"u{i}", name=f"u{i}", bufs=1)
                            for i in range(2)]
                    for dt in range(DT):
                        nc.tensor.matmul(g_ps[0][:], wg_t[:, dt, :], h2g[dt][:, 0:512],
                                         start=(dt == 0), stop=(dt == DT - 1))
                        nc.tensor.matmul(g_ps[1][:], wg_t[:, dt, :], h2g[dt][:, 512:1024],
                                         start=(dt == 0), stop=(dt == DT - 1))
                    for dt in range(DT):
                        nc.tensor.matmul(u_ps[0][:], wu_t[:, dt, :], h2g[dt][:, 0:512],
                                         start=(dt == 0), stop=(dt == DT - 1))
                        nc.tensor.matmul(u_ps[1][:], wu_t[:, dt, :], h2g[dt][:, 512:1024],
                                         start=(dt == 0), stop=(dt == DT - 1))
                    for i in range(2):
                        gs = s7.tile([128, 512], F32, tag="gs", bufs=2)
                        nc.scalar.activation(gs[:], g_ps[i][:], AF.Sigmoid)
                        nc.vector.tensor_tensor(
                            out=act_t[mt][:, hb + i * 512:hb + (i + 1) * 512],
                            in0=u_ps[i][:], in1=gs[:], op=OP.mult)
            # down over full s, lhsT reused x4; RS issued per 4-dt group
            for grp in range(4):
                for dt in range(grp * 4, grp * 4 + 4):
                    wd_t = s7.tile([128, MT, 128], F32R, tag="wd", bufs=2)
                    nc.sync.dma_start(wd_t[:], wdTt[:, :, dt, :])
                    d_ps = [ps7.tile([128, 512], F32, tag=f"d{i}", name=f"d{i}", bufs=1)
                            for i in range(4)]
                    for mt in range(MT):
                        for i in range(4):
                            nc.tensor.matmul(d_ps[i][:], wd_t[:, mt, :],
                                             act_t[mt][:, i * 512:(i + 1) * 512],
                                             start=(mt == 0), stop=(mt == MT - 1))
                    for i in range(4):
                        dn = s7.tile([128, 512], F32, tag="dn", bufs=3)
                        nc.vector.tensor_copy(dn[:], d_ps[i][:])
                        dl = (dt - grp * 4) * 128
                        nc.sync.dma_start(rs_in[grp][2 * i, dl:dl + 128, :],
                                          dn[:, 0:256])
                        nc.sync.dma_start(rs_in[grp][2 * i + 1, dl:dl + 128, :],
                                          dn[:, 256:512])
                nc.gpsimd.collective_compute("ReduceScatter", OP.add,
                                             ins=[rs_in[grp][:]], outs=[rs_out[grp][:]],
                                             replica_groups=RG)

        with tc.tile_pool(name="s8", bufs=1) as s8:
            for grp in range(4):
                for k in range(4):
                    dt = grp * 4 + k
                    rsb = s8.tile([128, SSH], F32, tag="rsb", bufs=3)
                    nc.sync.dma_start(rsb[:], rs_out[grp][k * 128:(k + 1) * 128, :])
                    fin = s8.tile([128, SSH], F32, tag="fin", bufs=3)
                    nc.vector.tensor_tensor(out=fin[:], in0=rsb[:], in1=res1[dt][:],
                                            op=OP.add)
                    nc.sync.dma_start(out_sh[dt * 128:(dt + 1) * 128, :], fin[:])

    nc.compile()
    return nc


_PROG = None


def _get_program():
    global _PROG
    if _PROG is None:
        _PROG = _build_program()
    return _PROG


def _prep_inputs(x, norm1_w, wq, wk, wv, wo, norm2_w, w_gate, w_up, w_down, cos, sin):
    x = np.asarray(x, dtype=np.float32)
    xTr = round_fp32r(np.ascontiguousarray(x.reshape(S, H).T))         # [H, S]
    cosT = np.ascontiguousarray(np.asarray(cos, np.float32).T)         # [HD, S]
    sinT = np.ascontiguousarray(np.asarray(sin, np.float32).T)
    n1 = np.asarray(norm1_w, np.float32)
    n2 = np.asarray(norm2_w, np.float32)
    wq = np.asarray(wq, np.float32) * n1[None, :] / np.sqrt(np.float32(HD))
    wk = np.asarray(wk, np.float32) * n1[None, :]
    wv = np.asarray(wv, np.float32) * n1[None, :]
    wg = np.asarray(w_gate, np.float32) * n2[None, :]
    wu = np.asarray(w_up, np.float32) * n2[None, :]
    wo = np.asarray(wo, np.float32)
    wd = np.asarray(w_down, np.float32)

    woT = round_fp32r(wo.T)                                            # [e=H, d=H]
    woTt = np.ascontiguousarray(
        woT.reshape(DT, 128, DT, 128).transpose(1, 0, 2, 3))           # [p, et, dt, c]

    in_maps = []
    for c in range(NC):
        e0 = c * EH
        m0 = c * MSH
        wqT = round_fp32r(wq[e0:e0 + EH, :].T)                         # [H, EH]
        wkT = round_fp32r(wk[e0:e0 + EH, :].T)
        wvT = round_fp32r(wv[e0:e0 + EH, :].T)
        wgT = round_fp32r(wg[m0:m0 + MSH, :].T)                        # [H, MSH]
        wuT = round_fp32r(wu[m0:m0 + MSH, :].T)
        wdT = round_fp32r(wd[:, m0:m0 + MSH].T)                        # [MSH, H]
        in_maps.append({
            "xTr": xTr,
            "xTrs": np.ascontiguousarray(xTr[:, c * SSH:(c + 1) * SSH]),
            "cosT": cosT, "sinT": sinT,
            "wqT": np.ascontiguousarray(wqT),
            "wkT": np.ascontiguousarray(wkT),
            "wvT": np.ascontiguousarray(wvT),
            "woTt": woTt,
            "wgTt": np.ascontiguousarray(
                wgT.reshape(DT, 128, MT, 128).transpose(1, 0, 2, 3)),
            "wuTt": np.ascontiguousarray(
                wuT.reshape(DT, 128, MT, 128).transpose(1, 0, 2, 3)),
            "wdTt": np.ascontiguousarray(
                wdT.reshape(MT, 128, DT, 128).transpose(1, 0, 2, 3)),
        })
    return in_maps


def kernel(x, norm1_w, wq, wk, wv, wo, norm2_w, w_gate, w_up, w_down, cos, sin,
           _want_results=False):
    in_maps = _prep_inputs(x, norm1_w, wq, wk, wv, wo, norm2_w,
                           w_gate, w_up, w_down, cos, sin)
    prog = _get_program()
    res = run_bass_kernel_spmd(prog, in_maps, list(range(NC)))
    out = np.empty((B, S, H), dtype=np.float32)
    for c in range(NC):
        out[0, c * SSH:(c + 1) * SSH, :] = res.results[c]["out_sh"].T
    if _want_results:
        return out, res
    return out



# revision 14
# speedup vs baseline: 1.2857x; 1.2857x over previous
"""Trainium2 Bass kernel for nn_DeepseekLayer (dense transformer layer).

Sharding (8 cores): Megatron-style TP, bf16 datapath (fp32 PSUM accum).
  - qkv head-sharded (2 heads/core) over full S; x resident in SBUF (bf16).
  - rmsnorm1 folded into rope tables / V-scale (scaling commutes with the
    linear projections); rstd computed on device.
  - attention: transposed-softmax layout (scores [sk, sq]), exp on ACT in
    [128,1024] chunks, fast-reciprocal normalize, per-head AllToAll (bf16)
    switches attention output to token shards so o_proj needs no all-reduce.
  - o_proj + residual + rmsnorm2: token-sharded (256 tokens/core), fp32
    residual.
  - MLP: AllGather hidden (bf16, 2 token-half chunks pipelined under MLP)
    -> tensor-parallel gate/up/down (1024 ff dims/core, weights streamed)
    -> per-half bf16 ReduceScatter overlapped with the other half -> local
    residual add -> host gathers token shards.
"""
import numpy as np
from contextlib import ExitStack

import ml_dtypes
from concourse import bacc
import concourse.tile as tile
import concourse.mybir as mybir
from concourse.bass_utils import run_bass_kernel_spmd

F32 = mybir.dt.float32
BF = mybir.dt.bfloat16
AF = mybir.ActivationFunctionType
OP = mybir.AluOpType

H = 2048          # hidden
NH = 16           # heads
HD = 128          # head dim
MLP = 8192
S = 2048          # sequence
B = 1
EPS = 1e-6
NC = 8            # cores
HPC = NH // NC    # heads per core = 2
EH = HPC * HD     # qkv out dims per core = 256
MSH = MLP // NC   # mlp dims per core = 1024
SSH = S // NC     # tokens per shard = 256
RG = [list(range(NC))]
DT = H // 128     # 16 d-tiles
MT = MSH // 128   # 8 m-tiles per core
BF_NP = ml_dtypes.bfloat16

_LDW_PATCHED = False


def _enable_ldw_opt():
    """Compile our NEFF with walrus --enable-ldw-opt=true (elides redundant
    LDWEIGHTS; concourse's default is false)."""
    global _LDW_PATCHED
    if _LDW_PATCHED:
        return
    import concourse.bass_utils as _bu
    _orig = _bu.run_command

    def _patched(argv, **kw):
        argv = ["--enable-ldw-opt=true" if a == "--enable-ldw-opt=false" else a
                for a in argv]
        return _orig(argv, **kw)

    _bu.run_command = _patched
    _LDW_PATCHED = True


def _build_program():
    # NOTE: walrus --enable-ldw-opt rejects explicit InstLdweights (emitted for
    # bf16 matmuls); bf16 stationaries get FWL instead, so keep the default.
    nc = bacc.Bacc(trn_type="TRN2", target_bir_lowering=False, debug=False,
                   num_devices=NC)

    def inp(name, shape, dt):
        return nc.dram_tensor(name, shape, dt, kind="ExternalInput").ap()

    xT = inp("xT", [H, S], BF)                  # x.T (feature-major), bf16
    xTrs = inp("xTrs", [H, SSH], F32)           # this core's token-shard, f32
    cosT = inp("cosT", [HD, S], F32)
    sinTs = inp("sinTs", [HD, S], F32)          # sin, rows 0:63 pre-negated
    wqkvT = inp("wqkvT", [H, 6 * 128], BF)      # cols: q0,q1,k0,k1,v0,v1
    woTt = inp("woTt", [128, DT, DT, 128], BF)  # wo.T tiled [p, et, dt, c]
    wgTt = inp("wgTt", [128, DT, MT, 128], BF)  # (wg*n2w).T shard [p, dt, mt, c]
    wuTt = inp("wuTt", [128, DT, MT, 128], BF)
    wdTt = inp("wdTt", [128, MT, DT, 128], BF)  # wd shard.T tiled [p, mt, dt, c]
    out_sh = nc.dram_tensor("out_sh", [H, SSH], F32, kind="ExternalOutput").ap()

    with tile.TileContext(nc) as tc, ExitStack() as top:
        dram = top.enter_context(tc.tile_pool(name="dram", bufs=1, space="DRAM"))
        per = top.enter_context(tc.tile_pool(name="per", bufs=1))
        ones_f = per.tile([128, 1], F32)
        nc.gpsimd.memset(ones_f[:], 1.0)
        ones_b = per.tile([128, 1], BF)
        nc.vector.tensor_copy(ones_b[:], ones_f[:])
        eps1 = per.tile([1, 1], F32)
        nc.gpsimd.memset(eps1[:], EPS)
        from concourse.masks import make_identity
        ident_f = per.tile([128, 128], F32)
        make_identity(nc, ident_f[:])
        ident_b = per.tile([128, 128], BF)
        nc.vector.tensor_copy(ident_b[:], ident_f[:])

        # ---- persistent SBUF: fp32 residual + attention I/O per head ----
        res_ctx = ExitStack()
        res_pool = res_ctx.enter_context(tc.tile_pool(name="res", bufs=1))
        res1 = [res_pool.tile([128, SSH], F32, name=f"res1_{dt}") for dt in range(DT)]
        xr = res_pool.tile([128, DT, SSH], F32, name="xr")
        for dt in range(DT):
            nc.sync.dma_start(xr[:, dt, :], xTrs[dt * 128:(dt + 1) * 128, :])

        qk_ctx = ExitStack()
        qk = qk_ctx.enter_context(tc.tile_pool(name="qk", bufs=1))
        qr = [qk.tile([128, S], BF, name=f"qr{h}") for h in range(HPC)]
        kr = [qk.tile([128, S], BF, name=f"kr{h}") for h in range(HPC)]
        V_sb = qk.tile([128, S // 128, EH], BF, name="V_sb")
        att = [qk.tile([128, S], BF, name=f"att{h}") for h in range(HPC)]

        a2a_in = [dram.tile([NC, 128, SSH], BF, name=f"a2a_in{h}") for h in range(HPC)]
        a2a_out = [dram.tile([NC, 128, SSH], BF, name=f"a2a_out{h}") for h in range(HPC)]

        # x / tables / qkv weights: live through P1+P2
        x_ctx = ExitStack()
        xp = x_ctx.enter_context(tc.tile_pool(name="xp", bufs=1))
        x_sb = xp.tile([128, DT, S], BF, name="x_sb")
        wqkv_sb = xp.tile([128, DT, 6 * 128], BF, name="wqkv_sb")
        cs_c = xp.tile([HD, S], F32, name="cs_c")     # cos * rstd
        cs_s = xp.tile([HD, S], F32, name="cs_s")     # (+-)sin * rstd
        rstd_bc = xp.tile([128, S], F32, name="rstd_bc")

        def rope(pool, dst, ps, c0):
            # dst[:, c0:c0+1024] = rotate-half rope on ps, rstd folded in cs_*
            mc = pool.tile([128, 1024], F32, tag="mc", bufs=1)
            msw = pool.tile([128, 1024], F32, tag="msw", bufs=1)
            nc.vector.tensor_tensor(out=mc[:], in0=ps[:],
                                    in1=cs_c[:, c0:c0 + 1024], op=OP.mult)
            nc.vector.tensor_tensor(out=msw[0:64, :], in0=ps[64:128, :],
                                    in1=cs_s[0:64, c0:c0 + 1024], op=OP.mult)
            nc.vector.tensor_tensor(out=msw[64:128, :], in0=ps[0:64, :],
                                    in1=cs_s[64:128, c0:c0 + 1024], op=OP.mult)
            nc.vector.tensor_tensor(out=dst[:, c0:c0 + 1024], in0=mc[:],
                                    in1=msw[:], op=OP.add)

        def qkv_oc(pool, pspool, h, kind, tag):
            oc = {"q": 0, "k": 2, "v": 4}[kind] + h
            for half in range(2):
                c0 = half * 1024
                ps = pspool.tile([128, 1024], F32, tag=tag, name=f"{kind}{h}_{half}",
                                 bufs=2)
                for dt in range(DT):
                    for i in range(2):
                        nc.tensor.matmul(
                            ps[:, i * 512:(i + 1) * 512],
                            wqkv_sb[:, dt, oc * 128:(oc + 1) * 128],
                            x_sb[:, dt, c0 + i * 512:c0 + (i + 1) * 512],
                            start=(dt == 0), stop=(dt == DT - 1))
                if kind == "q":
                    rope(pool, qr[h], ps[:], c0)
                elif kind == "k":
                    rope(pool, kr[h], ps[:], c0)
                else:
                    vsc = pool.tile([128, 1024], BF, tag="vsc", bufs=2)
                    nc.vector.tensor_tensor(out=vsc[:], in0=ps[:],
                                            in1=rstd_bc[:, c0:c0 + 1024], op=OP.mult)
                    for j in range(8):
                        kt = half * 8 + j
                        tr = pspool.tile([128, 128], BF, tag=tag,
                                         name=f"tr{h}{kt}", bufs=2)
                        nc.tensor.transpose(tr[:], vsc[:, j * 128:(j + 1) * 128],
                                            ident_b[:])
                        nc.vector.tensor_copy(V_sb[:, kt, h * 128:(h + 1) * 128], tr[:])

        # ============ P1: x load + rmsnorm1 stats + qkv head0 ============
        p1_ctx = ExitStack()
        s1 = p1_ctx.enter_context(tc.tile_pool(name="s1", bufs=1))
        ps1 = p1_ctx.enter_context(tc.tile_pool(name="ps1", bufs=1, space="PSUM"))

        nc.sync.dma_start(cs_c[:], cosT)
        nc.sync.dma_start(cs_s[:], sinTs)

        ss_ps = ps1.tile([1, S], F32, tag="stat", name="ss_ps", bufs=1)
        for dt in range(DT):
            nc.sync.dma_start(x_sb[:, dt, :], xT[dt * 128:(dt + 1) * 128, :])
            nc.sync.dma_start(wqkv_sb[:, dt, :], wqkvT[dt * 128:(dt + 1) * 128, :])
            for i in range(2):
                sq = s1.tile([128, 1024], BF, tag="sq", bufs=2)
                nc.vector.tensor_tensor(out=sq[:],
                                        in0=x_sb[:, dt, i * 1024:(i + 1) * 1024],
                                        in1=x_sb[:, dt, i * 1024:(i + 1) * 1024],
                                        op=OP.mult)
                for j in range(2):
                    c = i * 1024 + j * 512
                    nc.tensor.matmul(ss_ps[:, c:c + 512], ones_b[:],
                                     sq[:, j * 512:(j + 1) * 512],
                                     start=(dt == 0), stop=(dt == DT - 1))
        # rstd = 1/sqrt(mean+eps); broadcast; fold into rope tables
        for i in range(4):
            sdc = s1.tile([1, 512], F32, tag="sdc", bufs=2)
            nc.scalar.activation(sdc[:], ss_ps[:, i * 512:(i + 1) * 512], AF.Sqrt,
                                 bias=eps1[:], scale=1.0 / H)
            rsc = s1.tile([1, 512], F32, tag="rsc", bufs=2)
            nc.vector.reciprocal_approx_fast(out=rsc[:], in_=sdc[:])
            nc.gpsimd.partition_broadcast(rstd_bc[:, i * 512:(i + 1) * 512], rsc[:],
                                          channels=128)
        # fold rstd into the rope tables in place
        nc.vector.tensor_tensor(out=cs_c[:], in0=cs_c[:], in1=rstd_bc[:], op=OP.mult)
        nc.vector.tensor_tensor(out=cs_s[:], in0=cs_s[:], in1=rstd_bc[:], op=OP.mult)

        qkv_oc(s1, ps1, 0, "q", "qk")
        qkv_oc(s1, ps1, 0, "k", "qk")
        qkv_oc(s1, ps1, 0, "v", "qk")
        p1_ctx.close()

        # ============ P2: attention h0, qkv h1, attention h1 ============
        p2_ctx = ExitStack()
        s2 = p2_ctx.enter_context(tc.tile_pool(name="s2", bufs=1))
        ps2 = p2_ctx.enter_context(tc.tile_pool(name="ps2", bufs=1, space="PSUM"))

        def attn_head(h):
            for sc in range(2):          # sq chunks of 1024
                c0 = sc * 1024
                av = ps2.tile([128, 1024], F32, tag="av", name=f"av{h}{sc}", bufs=1)
                sm = ps2.tile([1, 1024], F32, tag="sm", name=f"sm{h}{sc}", bufs=1)
                for kt in range(DT):
                    st = ps2.tile([128, 1024], F32, tag="big", name=f"st{h}{sc}",
                                  bufs=2)
                    for i in range(2):
                        nc.tensor.matmul(st[:, i * 512:(i + 1) * 512],
                                         kr[h][:, kt * 128:(kt + 1) * 128],
                                         qr[h][:, c0 + i * 512:c0 + (i + 1) * 512],
                                         start=True, stop=True)
                    e = s2.tile([128, 1024], BF, tag="e", bufs=3)
                    nc.scalar.activation(e[:], st[:], AF.Exp)
                    for i in range(2):
                        nc.tensor.matmul(sm[:, i * 512:(i + 1) * 512], ones_b[:],
                                         e[:, i * 512:(i + 1) * 512],
                                         start=(kt == 0), stop=(kt == DT - 1))
                        nc.tensor.matmul(av[:, i * 512:(i + 1) * 512],
                                         V_sb[:, kt, h * 128:(h + 1) * 128],
                                         e[:, i * 512:(i + 1) * 512],
                                         start=(kt == 0), stop=(kt == DT - 1))
                rs = s2.tile([1, 1024], F32, tag="rs", bufs=1)
                nc.vector.reciprocal_approx_fast(out=rs[:], in_=sm[:])
                bc = s2.tile([128, 1024], F32, tag="bc", bufs=1)
                nc.gpsimd.partition_broadcast(bc[:], rs[:], channels=128)
                nc.vector.tensor_tensor(out=att[h][:, c0:c0 + 1024], in0=av[:],
                                        in1=bc[:], op=OP.mult)
            for j in range(NC):
                nc.sync.dma_start(a2a_in[h][j], att[h][:, j * SSH:(j + 1) * SSH])
            nc.gpsimd.collective_compute("AllToAll", OP.bypass,
                                         ins=[a2a_in[h][:]], outs=[a2a_out[h][:]],
                                         replica_groups=RG)

        attn_head(0)
        qkv_oc(s2, ps2, 1, "q", "big")
        qkv_oc(s2, ps2, 1, "k", "big")
        qkv_oc(s2, ps2, 1, "v", "big")
        attn_head(1)
        p2_ctx.close()
        x_ctx.close()
        qk_ctx.close()

        # ============ P3: o_proj + residual + rmsnorm2 + AG ============
        h2_ctx = ExitStack()
        h2p = h2_ctx.enter_context(tc.tile_pool(name="h2p", bufs=1))
        h2 = [h2p.tile([128, SSH], BF, name=f"h2_{dt}") for dt in range(DT)]
        ag_in = [dram.tile([128, DT, 128], BF, name=f"ag_in{t}") for t in range(2)]
        ag_out = [dram.tile([NC, 128, DT, 128], BF, addr_space="Shared",
                            name=f"ag_out{t}") for t in range(2)]
        with tc.tile_pool(name="s3", bufs=1) as s3, \
             tc.tile_pool(name="ps3", bufs=1, space="PSUM") as ps3:
            attg = s3.tile([128, DT, SSH], BF, tag="attg")
            for j in range(NC):
                for h in range(HPC):
                    nc.sync.dma_start(attg[:, 2 * j + h, :], a2a_out[h][j])
            ss2_ps = ps3.tile([1, SSH], F32, tag="ss2", name="ss2_ps")
            for dt in range(DT):
                wo_t = s3.tile([128, DT, 128], BF, tag="wo", bufs=3)
                nc.sync.dma_start(wo_t[:], woTt[:, :, dt, :])
                o_ps = ps3.tile([128, SSH], F32, tag="o", name="o_ps", bufs=3)
                for et in range(DT):
                    nc.tensor.matmul(o_ps[:], wo_t[:, et, :], attg[:, et, :],
                                     start=(et == 0), stop=(et == DT - 1))
                nc.vector.tensor_tensor(out=res1[dt][:], in0=o_ps[:],
                                        in1=xr[:, dt, :], op=OP.add)
                sq2 = s3.tile([128, SSH], BF, tag="sq2", bufs=2)
                nc.vector.tensor_tensor(out=sq2[:], in0=res1[dt][:], in1=res1[dt][:],
                                        op=OP.mult)
                nc.tensor.matmul(ss2_ps[:], ones_b[:], sq2[:],
                                 start=(dt == 0), stop=(dt == DT - 1))
            sd2 = s3.tile([1, SSH], F32, name="sd2")
            nc.scalar.activation(sd2[:], ss2_ps[:], AF.Sqrt, bias=eps1[:],
                                 scale=1.0 / H)
            rstd2 = s3.tile([1, SSH], F32, name="rstd2")
            nc.vector.reciprocal_approx_fast(out=rstd2[:], in_=sd2[:])
            rstd2_bc = s3.tile([128, SSH], F32, name="rstd2_bc")
            nc.gpsimd.partition_broadcast(rstd2_bc[:], rstd2[:], channels=128)
            for dt in range(DT):
                nc.vector.tensor_tensor(out=h2[dt][:], in0=res1[dt][:],
                                        in1=rstd2_bc[:], op=OP.mult)
            # AG per token-half so MLP(ts0) can start while AG(ts1) flies
            for ts in range(2):
                for dt in range(DT):
                    nc.sync.dma_start(ag_in[ts][:, dt, :],
                                      h2[dt][:, ts * 128:(ts + 1) * 128])
                nc.gpsimd.collective_compute("AllGather", OP.bypass,
                                             ins=[ag_in[ts][:]], outs=[ag_out[ts][:]],
                                             replica_groups=RG)

        # ============ P4: MLP per token-half, chunked RS ============
        rs_in = [dram.tile([NC, 128, DT, 128], BF, name=f"rs_in{t}") for t in range(2)]
        rs_out = [dram.tile([128, DT, 128], BF, name=f"rs_out{t}") for t in range(2)]
        with tc.tile_pool(name="s4", bufs=1) as s4, \
             tc.tile_pool(name="ps4", bufs=1, space="PSUM") as ps4:
            for ts in range(2):
                h2g = s4.tile([128, DT, 1024], BF, tag="h2g", bufs=2, name=f"h2g{ts}")
                for j in range(NC):
                    nc.sync.dma_start(h2g[:, :, j * 128:(j + 1) * 128], ag_out[ts][j])
                act_t = s4.tile([128, MT, 1024], BF, tag="act", bufs=1,
                                name=f"act{ts}")
                for mt in range(MT):
                    wg_t = s4.tile([128, DT, 128], BF, tag="wg", bufs=2)
                    wu_t = s4.tile([128, DT, 128], BF, tag="wu", bufs=2)
                    nc.sync.dma_start(wg_t[:], wgTt[:, :, mt, :])
                    nc.sync.dma_start(wu_t[:], wuTt[:, :, mt, :])
                    g_ps = ps4.tile([128, 1024], F32, tag="g", name=f"g{ts}{mt}",
                                    bufs=2)
                    u_ps = ps4.tile([128, 1024], F32, tag="u", name=f"u{ts}{mt}",
                                    bufs=2)
                    for dt in range(DT):
                        for i in range(2):
                            nc.tensor.matmul(
                                g_ps[:, i * 512:(i + 1) * 512], wg_t[:, dt, :],
                                h2g[:, dt, i * 512:(i + 1) * 512],
                                start=(dt == 0), stop=(dt == DT - 1))
                    for dt in range(DT):
                        for i in range(2):
                            nc.tensor.matmul(
                                u_ps[:, i * 512:(i + 1) * 512], wu_t[:, dt, :],
                                h2g[:, dt, i * 512:(i + 1) * 512],
                                start=(dt == 0), stop=(dt == DT - 1))
                    gs = s4.tile([128, 1024], BF, tag="gs", bufs=2)
                    nc.scalar.activation(gs[:], g_ps[:], AF.Sigmoid)
                    nc.vector.tensor_tensor(out=act_t[:, mt, :], in0=u_ps[:],
                                            in1=gs[:], op=OP.mult)
                # down: contraction over mt, outputs [d, 1024] per dt
                dn_all = s4.tile([128, DT, 1024], BF, tag="dn", bufs=1,
                                 name=f"dn{ts}")
                for dt in range(DT):
                    wd_t = s4.tile([128, MT, 128], BF, tag="wd", bufs=3)
                    nc.sync.dma_start(wd_t[:], wdTt[:, :, dt, :])
                    d_ps = ps4.tile([128, 1024], F32,
                                    tag=("g" if dt % 2 == 0 else "u"),
                                    name=f"d{ts}{dt}", bufs=2)
                    for mt in range(MT):
                        for i in range(2):
                            nc.tensor.matmul(d_ps[:, i * 512:(i + 1) * 512],
                                             wd_t[:, mt, :],
                                             act_t[:, mt, i * 512:(i + 1) * 512],
                                             start=(mt == 0), stop=(mt == MT - 1))
                    nc.vector.tensor_copy(dn_all[:, dt, :], d_ps[:])
                for j in range(NC):
                    nc.sync.dma_start(rs_in[ts][j],
                                      dn_all[:, :, j * 128:(j + 1) * 128])
                nc.gpsimd.collective_compute("ReduceScatter", OP.add,
                                             ins=[rs_in[ts][:]], outs=[rs_out[ts][:]],
                                             replica_groups=RG)

        with tc.tile_pool(name="s5", bufs=1) as s5:
            for ts in range(2):
                rsb = s5.tile([128, DT, 128], BF, tag="rsb", bufs=2)
                nc.sync.dma_start(rsb[:], rs_out[ts][:])
                for dt in range(DT):
                    fin = s5.tile([128, 128], F32, tag="fin", bufs=4)
                    nc.vector.tensor_tensor(out=fin[:], in0=rsb[:, dt, :],
                                            in1=res1[dt][:, ts * 128:(ts + 1) * 128],
                                            op=OP.add)
                    nc.sync.dma_start(out_sh[dt * 128:(dt + 1) * 128,
                                             ts * 128:(ts + 1) * 128], fin[:])

        h2_ctx.close()
        res_ctx.close()

    nc.compile()
    return nc


_PROG = None


def _get_program():
    global _PROG
    if _PROG is None:
        _PROG = _build_program()
    return _PROG


def _prep_inputs(x, norm1_w, wq, wk, wv, wo, norm2_w, w_gate, w_up, w_down, cos, sin):
    x = np.asarray(x, dtype=np.float32)
    xT = np.ascontiguousarray(x.reshape(S, H).T)                       # [H, S]
    xT_bf = xT.astype(BF_NP)
    cosT = np.ascontiguousarray(np.asarray(cos, np.float32).T)         # [HD, S]
    sinT = np.ascontiguousarray(np.asarray(sin, np.float32).T)
    sinTs = sinT.copy()
    sinTs[0:HD // 2] = -sinTs[0:HD // 2]       # rotate_half sign for lo rows
    n1 = np.asarray(norm1_w, np.float32)
    n2 = np.asarray(norm2_w, np.float32)
    wq = np.asarray(wq, np.float32) * n1[None, :] / np.sqrt(np.float32(HD))
    wk = np.asarray(wk, np.float32) * n1[None, :]
    wv = np.asarray(wv, np.float32) * n1[None, :]
    wg = np.asarray(w_gate, np.float32) * n2[None, :]
    wu = np.asarray(w_up, np.float32) * n2[None, :]
    wo = np.asarray(wo, np.float32)
    wd = np.asarray(w_down, np.float32)

    woT = wo.T.astype(BF_NP)                                           # [e=H, d=H]
    woTt = np.ascontiguousarray(
        woT.reshape(DT, 128, DT, 128).transpose(1, 0, 2, 3))           # [p, et, dt, c]

    in_maps = []
    for c in range(NC):
        e0 = c * EH
        m0 = c * MSH
        wqkv = np.concatenate([wq[e0:e0 + EH, :], wk[e0:e0 + EH, :],
                               wv[e0:e0 + EH, :]], axis=0)             # [768, H]
        wqkvT = np.ascontiguousarray(wqkv.T).astype(BF_NP)             # [H, 768]
        wgT = wg[m0:m0 + MSH, :].T.astype(BF_NP)                       # [H, MSH]
        wuT = wu[m0:m0 + MSH, :].T.astype(BF_NP)
        wdT = wd[:, m0:m0 + MSH].T.astype(BF_NP)                       # [MSH, H]
        in_maps.append({
            "xT": xT_bf,
            "xTrs": np.ascontiguousarray(xT[:, c * SSH:(c + 1) * SSH]),
            "cosT": cosT, "sinTs": sinTs,
            "wqkvT": wqkvT,
            "woTt": woTt,
            "wgTt": np.ascontiguousarray(
                wgT.reshape(DT, 128, MT, 128).transpose(1, 0, 2, 3)),
            "wuTt": np.ascontiguousarray(
                wuT.reshape(DT, 128, MT, 128).transpose(1, 0, 2, 3)),
            "wdTt": np.ascontiguousarray(
                wdT.reshape(MT, 128, DT, 128).transpose(1, 0, 2, 3)),
        })
    return in_maps


def kernel(x, norm1_w, wq, wk, wv, wo, norm2_w, w_gate, w_up, w_down, cos, sin,
           _want_results=False):
    in_maps = _prep_inputs(x, norm1_w, wq, wk, wv, wo, norm2_w,
                           w_gate, w_up, w_down, cos, sin)
    prog = _get_program()
    res = run_bass_kernel_spmd(prog, in_maps, list(range(NC)))
    out = np.empty((B, S, H), dtype=np.float32)
    for c in range(NC):
        out[0, c * SSH:(c + 1) * SSH, :] = res.results[c]["out_sh"].T
    if _want_results:
        return out, res
    return out
